# revision 1
# baseline (speedup 1.0000x reference)
"""Trainium2 Bass kernel for nn_FCVI_Net_78864189489850.

Computation (reference):
  L = lower-tri scatter of cov_vector (exp on diag)          [769, 769]
  samples = mean + L @ z                                      [769, S, B]
  W0 = samples[0:256], b0 = samples[256:512],
  W1 = samples[512:768], b1 = samples[768]
  h = relu(x * W0 + b0);  out = sum_o h * W1 + b1             [S, B]

Strategy (8 NeuronCores, batch-sharded, no cross-device comms):
  - Host builds L (cheap scatter + 769 exps), transposes to LT, casts to
    f16.  Each core gets a B-shard of z (columns c = s*256 + b_local,
    4096 cols) in f16 — halves HBM traffic; f16 matmul runs at full rate.
  - Transposed-orientation matmuls: sT[c, i] = sum_k z[k,c] * LT[k,i]
    with triangular k-tile skip.  Per 128-column tile, 10 matmuls:
      pA[:, 0:512]   <- k-tiles 0,1 (N=512; merged i-ranges)
      pA[:, 256:512] <- k-tiles 2,3 (N=256)
      pB[:, 0:257]   <- k-tiles 0..5 (N=257, i in [512, 769))
  - All mean terms ride on DVE constants (a = x*mean0 + mean1 per batch
    parity, m2b broadcast), the b1 row's k=768 term + mean768 comes in
    via a host-precomputed affine z8p[p, m] = L[768,768]*z[768,c] +
    mean[768].
  - Per c-tile: ACT scales sT0 by per-partition x, DVE adds sT1 + a,
    ACT applies relu, DVE multiplies by (sT2 + m2b) and row-reduces via
    scalar_tensor_tensor's accumulator.  Output staged [128, 32];
    host reassembles [16, 2048].
"""
import os
import numpy as np

P = 769
S = 16
B = 2048
NCORES = 8
BC = B // NCORES          # 256 batch per core
NCOL = S * BC             # 4096 columns per core
NCT = NCOL // 128         # 32 c-tiles per core
NCHUNK = 8                # z DMA chunks
CHW = NCOL // NCHUNK      # 512

_cache = {}


def _mm_dtype():
    import concourse.mybir as mybir
    name = os.environ.get("BASS_FCVI_DTYPE", "f16")
    return {
        "f16": (mybir.dt.float16, np.float16),
        "f32r": (mybir.dt.float32r, np.float32),
    }[name]


def _build_program():
    import concourse.bacc as bacc
    import concourse.tile as tile
    from concourse import mybir

    mmdt, _ = _mm_dtype()
    f32 = mybir.dt.float32

    nc = bacc.Bacc("TRN2", target_bir_lowering=False, debug=False)

    za_d = nc.dram_tensor("za", [768, NCOL], mmdt, kind="ExternalInput")
    lt_d = nc.dram_tensor("lt", [768, P], mmdt, kind="ExternalInput")
    cst_d = nc.dram_tensor("cst", [128, 802], f32, kind="ExternalInput")
    out_d = nc.dram_tensor("out", [128, NCT], f32, kind="ExternalOutput")

    # LT column ranges stored per k-tile (LT[k, i] == 0 for i < k)
    LT_COLS = [(0, 769), (0, 769), (256, 769), (256, 769),
               (512, 769), (512, 769)]

    with tile.TileContext(nc) as tc:
        with (
            tc.tile_pool(name="zpool", bufs=1) as zpool,
            tc.tile_pool(name="ltpool", bufs=1) as ltpool,
            tc.tile_pool(name="cpool", bufs=1) as cpool,
            tc.tile_pool(name="work", bufs=4) as work,
            tc.tile_pool(name="gsc", bufs=3) as gsc,
            tc.tile_pool(name="pa", bufs=3, space="PSUM") as pa_pool,
            tc.tile_pool(name="pb", bufs=4, space="PSUM") as pb_pool,
        ):
            # --- DMAs, ordered so c-tile 0's dependencies land first ---
            ltt = [None] * 6
            zc = [None] * NCHUNK

            def load_lt(t):
                lo, hi = LT_COLS[t]
                tl = ltpool.tile([128, hi - lo], mmdt, tag=f"lt{t}")
                nc.sync.dma_start(
                    out=tl[:], in_=lt_d.ap()[t * 128:(t + 1) * 128, lo:hi])
                ltt[t] = tl

            def load_zc(q):
                zq = zpool.tile([128, 6, CHW], mmdt, tag=f"zc{q}")
                src = za_d.ap()[:, q * CHW:(q + 1) * CHW].rearrange(
                    "(t p) c -> p t c", p=128)
                nc.sync.dma_start(out=zq[:], in_=src)
                zc[q] = zq

            zc0h = []
            for h_ in range(2):
                zq = zpool.tile([128, 6, CHW // 2], mmdt, tag=f"zc0{h_}")
                src = za_d.ap()[:, h_ * (CHW // 2):(h_ + 1) * (CHW // 2)].rearrange(
                    "(t p) c -> p t c", p=128)
                nc.sync.dma_start(out=zq[:], in_=src)
                zc0h.append(zq)
            load_lt(0)
            load_lt(1)

            cst = cpool.tile([128, 802], f32, tag="cst")
            nc.sync.dma_start(out=cst[:], in_=cst_d.ap()[:, :])
            m01 = cst[:, 0:512]
            m2b = cst[:, 512:768]
            z8p = cst[:, 768:800]
            xvt = cst[:, 800:802]

            load_lt(2)
            load_lt(3)
            load_zc(1)
            load_lt(4)
            load_lt(5)
            for q in range(2, NCHUNK):
                load_zc(q)

            apar = []
            for par in range(2):
                a = cpool.tile([128, 256], f32, tag=f"a{par}")
                nc.vector.scalar_tensor_tensor(
                    out=a[:], in0=m01[:, 0:256], scalar=xvt[:, par:par + 1],
                    in1=m01[:, 256:512],
                    op0=mybir.AluOpType.mult, op1=mybir.AluOpType.add)
                apar.append(a)

            stag = cpool.tile([128, NCT], f32, tag="stag")
            s3stag = cpool.tile([128, NCT], f32, tag="s3stag")
            stag2 = cpool.tile([128, NCT], f32, tag="stag2")

            def rhs(t, g0, g1):
                lo, _ = LT_COLS[t]
                return ltt[t][:, g0 - lo:g1 - lo]

            for m in range(NCT):
                q, cl = divmod(m * 128, CHW)

                def lhsT(t):
                    if q == 0:
                        return zc0h[m // 2][:, t, (m % 2) * 128:(m % 2) * 128 + 128]
                    return zc[q][:, t, cl:cl + 128]

                pA = pa_pool.tile([128, 512], f32, tag="pA")
                pB = pb_pool.tile([128, 257], f32, tag="pB")
                # k-tiles 0,1 cover i in [0, 512) in one N=512 matmul each
                nc.tensor.matmul(pA[:, 0:512], lhsT(0), rhs(0, 0, 512),
                                 start=True, stop=False)
                nc.tensor.matmul(pA[:, 0:512], lhsT(1), rhs(1, 0, 512),
                                 start=False, stop=False)
                # k-tiles 2,3 only contribute to i in [256, 512)
                nc.tensor.matmul(pA[:, 256:512], lhsT(2), rhs(2, 256, 512),
                                 start=False, stop=False)
                nc.tensor.matmul(pA[:, 256:512], lhsT(3), rhs(3, 256, 512),
                                 start=False, stop=True)
                # i in [512, 769): k-tiles 0..5
                for t in range(6):
                    nc.tensor.matmul(pB[:, 0:257], lhsT(t), rhs(t, 512, 769),
                                     start=(t == 0), stop=(t == 5))

                # t = x * sT0   (ACT: PSUM->SBUF copy with per-partition scale)
                t_ = work.tile([128, 256], f32, tag="t")
                nc.scalar.activation(t_[:], pA[:, 0:256],
                                     mybir.ActivationFunctionType.Copy,
                                     scale=xvt[:, m % 2:m % 2 + 1])
                # u = t + sT1
                u = work.tile([128, 256], f32, tag="u")
                nc.vector.tensor_add(u[:], t_[:], pA[:, 256:512])
                # u2 = u + (x*mean0 + mean1)
                u2 = work.tile([128, 256], f32, tag="u2")
                nc.vector.tensor_add(u2[:], u[:], apar[m % 2][:])
                # h = relu(u2)
                h = work.tile([128, 256], f32, tag="h")
                nc.scalar.activation(h[:], u2[:],
                                     mybir.ActivationFunctionType.Relu)
                # v = sT2 + mean2
                v = work.tile([128, 256], f32, tag="v")
                nc.vector.tensor_add(v[:], pB[:, 0:256], m2b)
                # g = h * v ; stag[:, m] = sum_o g
                g = gsc.tile([128, 256], f32, tag="g")
                nc.vector.scalar_tensor_tensor(
                    out=g[:], in0=h[:], scalar=1.0, in1=v[:],
                    op0=mybir.AluOpType.mult, op1=mybir.AluOpType.mult,
                    accum_out=stag[:, m:m + 1])
                # b1: s3stag[:, m] = (sum_{k<768} L[768,k] z[k,c]) + z8p
                nc.vector.tensor_add(s3stag[:, m:m + 1], pB[:, 256:257],
                                     z8p[:, m:m + 1])

                if m in (NCT // 2 - 1, NCT - 1):
                    h_ = 0 if m == NCT // 2 - 1 else 1
                    sl = slice(h_ * (NCT // 2), (h_ + 1) * (NCT // 2))
                    nc.vector.tensor_add(stag2[:, sl], stag[:, sl],
                                         s3stag[:, sl])
                    nc.sync.dma_start(out=out_d.ap()[:, sl], in_=stag2[:, sl])

    nc.compile()
    return nc


def _prep_inputs(x, mean, cov_vector, z):
    _, npdt = _mm_dtype()

    L = np.zeros((P, P), dtype=np.float32)
    L[np.tril_indices(P)] = cov_vector
    d = np.diag(L).copy()
    L[np.diag_indices(P)] = np.exp(d)

    lt = np.ascontiguousarray(L.T[:768]).astype(npdt)     # rows k in [0, 768)

    cst_base = np.empty((128, 802), dtype=np.float32)
    cst_base[:, 0:512] = mean[None, 0:512]
    cst_base[:, 512:768] = mean[None, 512:768]


    z2 = z.reshape(P, S, B)
    in_maps = []
    for c in range(NCORES):
        zs = z2[:, :, c * BC:(c + 1) * BC].reshape(P, NCOL)
        za = zs[:768].astype(npdt)
        # z8p[p, m] = L[768,768] * z[768, 128m + p] + mean[768]
        z8 = zs[768].astype(np.float32)                    # [4096]
        cst = cst_base.copy()
        cst[:, 768:800] = (L[768, 768] * z8 + mean[768]).reshape(NCT, 128).T
        xs = x[c * BC:(c + 1) * BC]
        cst[:, 800] = xs[0:128]
        cst[:, 801] = xs[128:256]
        in_maps.append({"za": np.ascontiguousarray(za), "lt": lt,
                        "cst": cst})
    return in_maps


def _assemble(results):
    out = np.empty((S, B), dtype=np.float32)
    for c in range(NCORES):
        o = results[c]["out"]                       # [128, 32]
        oc = o.reshape(128, S, 2).transpose(1, 2, 0).reshape(S, BC)
        out[:, c * BC:(c + 1) * BC] = oc
    return out


def _run(inputs, trace=False, trace_kwargs=None):
    from concourse.bass_utils import run_bass_kernel_spmd

    key = os.environ.get("BASS_FCVI_DTYPE", "f16")
    if key not in _cache:
        _cache[key] = _build_program()
    nc = _cache[key]

    in_maps = _prep_inputs(**inputs)
    kw = {}
    if trace:
        kw["trace"] = True
        if trace_kwargs:
            kw.update(trace_kwargs)
    res = run_bass_kernel_spmd(nc, in_maps, core_ids=list(range(NCORES)), **kw)
    return _assemble(res.results), res


def kernel(x, mean, cov_vector, z):
    out, _ = _run(dict(x=np.asarray(x), mean=np.asarray(mean),
                       cov_vector=np.asarray(cov_vector), z=np.asarray(z)))
    return out



# revision 2
# speedup vs baseline: 1.0279x; 1.0279x over previous
"""Trainium2 Bass kernel for nn_FCVI_Net_78864189489850.

Computation (reference):
  L = lower-tri scatter of cov_vector (exp on diag)           [769, 769]
  samples = mean + L @ z                                      [769, S, B]
  W0 = samples[0:256], b0 = samples[256:512],
  W1 = samples[512:768], b1 = samples[768]
  out = sum_o relu(x*W0 + b0) * W1 + b1                       [S, B]

Strategy (8 NeuronCores, batch-sharded, no cross-device comms):
  - Everything is fused into ONE PSUM bank per 128-column tile:
      pub[:, 0:256]   = u2 = x*s0 + s1          (via host-prescaled xz rows)
      pub[:, 256:512] = v  = s2 + mean2         (mean2 via a K=1 ones matmul)
  - 9 matmuls per c-tile with block-triangular trimming (2944 cycles):
      z-t0..t2 -> N=512, z-t3 -> N=384, xz-t0 -> N=256, xz-t1 -> N=128,
      ones x mean2 -> N=256, z-t4 -> N=256, z-t5 -> N=128.
  - DVE: u2a = pub0 + apar (apar = x*mean0+mean1, host-built, 2 parities),
         g = max(u2a, 0) * pub1 with accum_out -> stag column  (relu fused!)
  - b1 row (s3 = L[768,:]@z + mean[768]) is computed fully on host and
    added after the device pass; scalar engine and gpsimd are unused.
  - z ships as f16 in DMA-friendly [chunk][part][slot][col] layout
    (8KB contiguous per partition per chunk); LT ships as one packed
    [128, 2688] f16 image of exactly the needed column ranges.
  - ~8 junk warm-up matmuls at t=0 (no DMA deps) spin the PE HAM clock
    up to 2.4 GHz while the first z chunk streams in.
"""
import os
import numpy as np

P = 769
S = 16
B = 2048
NCORES = 8
BC = B // NCORES          # 256 batch per core
NCOL = S * BC             # 4096 columns per core
NCT = NCOL // 128         # 32 c-tiles per core
NCHUNK = 8                # z DMA chunks
CHW = NCOL // NCHUNK      # 512

NWARM = int(os.environ.get("BASS_FCVI_WARM", "8"))

# lt image segments: (k0, i0, i1) in MM issue order; offsets accumulate
LT_SEGS = [
    (0,   256, 768),   # z-t0  -> pub[:, 0:512]
    (128, 256, 768),   # z-t1  -> pub[:, 0:512]
    (256, 256, 768),   # z-t2  -> pub[:, 0:512]
    (384, 384, 768),   # z-t3  -> pub[:, 128:512]
    (512, 512, 768),   # z-t4  -> pub[:, 256:512]
    (640, 640, 768),   # z-t5  -> pub[:, 384:512]
    (0,   0,   256),   # xz-t0 -> pub[:, 0:256]
    (128, 128, 256),   # xz-t1 -> pub[:, 128:256]
]
LT_OFF = []
_o = 0
for _k0, _i0, _i1 in LT_SEGS:
    LT_OFF.append(_o)
    _o += _i1 - _i0
LT_W = _o  # 2688

_cache = {}


def _build_program():
    import concourse.bacc as bacc
    import concourse.tile as tile
    from concourse import mybir

    f16 = mybir.dt.float16
    f32 = mybir.dt.float32

    nc = bacc.Bacc("TRN2", target_bir_lowering=False, debug=False)

    za_d = nc.dram_tensor("za", [NCHUNK, 128, 8, CHW], f16, kind="ExternalInput")
    lt_d = nc.dram_tensor("lt", [128, LT_W], f16, kind="ExternalInput")
    ap_d = nc.dram_tensor("apm", [128, 2, 256], f32, kind="ExternalInput")
    m2_d = nc.dram_tensor("m2", [1, 256], f16, kind="ExternalInput")
    out_d = nc.dram_tensor("out", [128, NCT], f32, kind="ExternalOutput")

    with tile.TileContext(nc) as tc:
        with (
            tc.tile_pool(name="zpool", bufs=1) as zpool,
            tc.tile_pool(name="cpool", bufs=1) as cpool,
            tc.tile_pool(name="upool", bufs=3) as upool,
            tc.tile_pool(name="gpool", bufs=2) as gpool,
            tc.tile_pool(name="pub", bufs=7, space="PSUM") as pub_pool,
            tc.tile_pool(name="pwarm", bufs=1, space="PSUM") as pwarm_pool,
        ):
            # --- PE warm-up: junk matmuls with no DMA deps spin HAM to 2.4GHz
            warm = cpool.tile([128, 640], f16, tag="warm")
            nc.vector.memset(warm[:], 0.0)
            onesr = cpool.tile([1, 128], f16, tag="ones")
            nc.vector.memset(onesr[:], 1.0)
            pw = pwarm_pool.tile([128, 512], f32, tag="pw")
            for _ in range(NWARM):
                nc.tensor.matmul(pw[:], warm[:, 0:128], warm[:, 128:640],
                                 start=True, stop=True)

            # --- DMAs, ordered so c-tile 0's dependencies land first ---
            ltt = cpool.tile([128, LT_W], f16, tag="lt")
            nc.sync.dma_start(out=ltt[:], in_=lt_d.ap()[:, :])

            zc = [None] * NCHUNK
            zc0 = [None, None]
            for h in range(2):
                t = zpool.tile([128, 4, CHW], f16, tag=f"zc0{h}")
                nc.sync.dma_start(out=t[:], in_=za_d.ap()[0, :, h * 4:(h + 1) * 4, :])
                zc0[h] = t

            apm = cpool.tile([128, 2, 256], f32, tag="apm")
            nc.sync.dma_start(out=apm[:], in_=ap_d.ap()[:, :, :])
            m2t = cpool.tile([1, 256], f16, tag="m2")
            nc.sync.dma_start(out=m2t[:], in_=m2_d.ap()[:, :])

            for q in range(1, NCHUNK):
                t = zpool.tile([128, 8, CHW], f16, tag=f"zc{q}")
                nc.sync.dma_start(out=t[:], in_=za_d.ap()[q, :, :, :])
                zc[q] = t

            stag = cpool.tile([128, NCT], f32, tag="stag")

            def lhs(m, s):
                q, cl = divmod(m * 128, CHW)
                if q == 0:
                    return zc0[s // 4][:, s % 4, cl:cl + 128]
                return zc[q][:, s, cl:cl + 128]

            def seg(t):
                return ltt[:, LT_OFF[t]:LT_OFF[t] + (LT_SEGS[t][2] - LT_SEGS[t][1])]

            MM = nc.tensor.matmul
            for m in range(NCT):
                pub = pub_pool.tile([128, 512], f32, tag="pub")
                MM(pub[:, 0:512],   lhs(m, 0), seg(0), start=True, stop=False)
                MM(pub[:, 0:512],   lhs(m, 1), seg(1), start=False, stop=False)
                MM(pub[:, 0:512],   lhs(m, 2), seg(2), start=False, stop=False)
                MM(pub[:, 128:512], lhs(m, 3), seg(3), start=False, stop=False)
                MM(pub[:, 0:256],   lhs(m, 6), seg(6), start=False, stop=False)
                MM(pub[:, 128:256], lhs(m, 7), seg(7), start=False, stop=False)
                MM(pub[:, 256:512], onesr[:, :], m2t[:, :], start=False, stop=False)
                MM(pub[:, 256:512], lhs(m, 4), seg(4), start=False, stop=False)
                MM(pub[:, 384:512], lhs(m, 5), seg(5), start=False, stop=True)

                u2 = upool.tile([128, 256], f32, tag="u2")
                nc.vector.tensor_add(u2[:], pub[:, 0:256], apm[:, m % 2, :])
                g = gpool.tile([128, 256], f32, tag="g")
                nc.vector.scalar_tensor_tensor(
                    out=g[:], in0=u2[:], scalar=0.0, in1=pub[:, 256:512],
                    op0=mybir.AluOpType.max, op1=mybir.AluOpType.mult,
                    accum_out=stag[:, m:m + 1])

            nc.sync.dma_start(out=out_d.ap()[:, :], in_=stag[:])

    nc.compile()
    return nc


def _prep_inputs(x, mean, cov_vector, z):
    f16 = np.float16

    L = np.zeros((P, P), dtype=np.float32)
    L[np.tril_indices(P)] = cov_vector
    d = np.diag(L).copy()
    L[np.diag_indices(P)] = np.exp(d)
    LT = L.T  # lt[k, i] = L[i, k]

    ltimg = np.empty((128, LT_W), dtype=f16)
    for (k0, i0, i1), off in zip(LT_SEGS, LT_OFF):
        ltimg[:, off:off + (i1 - i0)] = LT[k0:k0 + 128, i0:i1]

    m2img = np.ascontiguousarray(mean[None, 512:768]).astype(f16)

    z2 = z.reshape(P, S, B)
    in_maps = []
    for c in range(NCORES):
        zs = z2[:, :, c * BC:(c + 1) * BC].reshape(P, NCOL)  # [769, 4096] f32
        xs = x[c * BC:(c + 1) * BC]                           # [256]
        xcol = np.tile(xs, S)                                 # x per column

        zap = np.empty((NCHUNK, 128, 8, CHW), dtype=f16)
        zap[:, :, 0:6, :] = (
            zs[:768].astype(f16).reshape(6, 128, NCHUNK, CHW).transpose(2, 1, 0, 3))
        xz = (xcol[None, :] * zs[0:256]).astype(f16)          # [256, 4096]
        zap[:, :, 6, :] = xz[0:128].reshape(128, NCHUNK, CHW).transpose(1, 0, 2)
        zap[:, :, 7, :] = xz[128:256].reshape(128, NCHUNK, CHW).transpose(1, 0, 2)

        apm = (xs.reshape(2, 128).T[:, :, None] * mean[None, None, 0:256]
               + mean[None, None, 256:512]).astype(np.float32)  # [128, 2, 256]

        # b1 row handled fully on host: s3[c] = L[768,:] @ z + mean[768]
        s3 = LT[:, 768] @ zs + mean[768]                       # [4096]
        s3img = s3.reshape(NCT, 128).T.astype(np.float32)      # [128, NCT]

        in_maps.append({"za": zap, "lt": ltimg, "apm": np.ascontiguousarray(apm),
                        "m2": m2img, "_s3": s3img})
    return in_maps


def _assemble(results, s3imgs):
    out = np.empty((S, B), dtype=np.float32)
    for c in range(NCORES):
        o = results[c]["out"] + s3imgs[c]                    # [128, 32]
        oc = o.reshape(128, S, 2).transpose(1, 2, 0).reshape(S, BC)
        out[:, c * BC:(c + 1) * BC] = oc
    return out


def _run(inputs, trace=False, trace_kwargs=None):
    from concourse.bass_utils import run_bass_kernel_spmd

    if "prog" not in _cache:
        _cache["prog"] = _build_program()
    nc = _cache["prog"]

    in_maps = _prep_inputs(**inputs)
    s3imgs = [im.pop("_s3") for im in in_maps]
    kw = {}
    if trace:
        kw["trace"] = True
        if trace_kwargs:
            kw.update(trace_kwargs)
    res = run_bass_kernel_spmd(nc, in_maps, core_ids=list(range(NCORES)), **kw)
    return _assemble(res.results, s3imgs), res


def kernel(x, mean, cov_vector, z):
    out, _ = _run(dict(x=np.asarray(x), mean=np.asarray(mean),
                       cov_vector=np.asarray(cov_vector), z=np.asarray(z)))
    return out


# revision 6
# speedup vs baseline: 1.1605x; 1.1290x over previous
"""Trainium2 Bass kernel for nn_FCVI_Net_78864189489850.

Computation (reference):
  L = lower-tri scatter of cov_vector (exp on diag)           [769, 769]
  samples = mean + L @ z                                      [769, S, B]
  W0 = samples[0:256], b0 = samples[256:512],
  W1 = samples[512:768], b1 = samples[768]
  out = sum_o relu(x*W0 + b0) * W1 + b1                       [S, B]

Strategy (8 NeuronCores, batch-sharded, no cross-device comms):
  - Everything is fused into ONE PSUM bank per 128-column tile:
      pub[:, 0:256]   = u2 = x*s0 + s1          (via host-prescaled xz rows)
      pub[:, 256:512] = v  = s2 + mean2         (mean2 via a K=1 ones matmul)
  - 9 matmuls per c-tile with block-triangular trimming (2944 cycles):
      z-t0..t2 -> N=512, z-t3 -> N=384, xz-t0 -> N=256, xz-t1 -> N=128,
      ones x mean2 -> N=256, z-t4 -> N=256, z-t5 -> N=128.
  - DVE: u2a = pub0 + apar (apar = x*mean0+mean1, host-built, 2 parities),
         g = max(u2a, 0) * pub1 with accum_out -> stag column  (relu fused!)
  - b1 row (s3 = L[768,:]@z + mean[768]) is computed fully on host and
    added after the device pass; scalar engine and gpsimd are unused.
  - z ships as f16 in DMA-friendly [chunk][part][slot][col] layout
    (8KB contiguous per partition per chunk); LT ships as one packed
    [128, 2688] f16 image of exactly the needed column ranges.
  - ~8 junk warm-up matmuls at t=0 (no DMA deps) spin the PE HAM clock
    up to 2.4 GHz while the first z chunk streams in.
"""
import os
import numpy as np

P = 769
S = 16
B = 2048
NCORES = 8
BC = B // NCORES          # 256 batch per core
NCOL = S * BC             # 4096 columns per core
NCT = NCOL // 128         # 32 c-tiles per core
NCHUNK = 8                # z DMA chunks
CHW = NCOL // NCHUNK      # 512

NWARM = int(os.environ.get("BASS_FCVI_WARM", "8"))

# lt image segments: (k0, i0, i1) in MM issue order; offsets accumulate
LT_SEGS = [
    (0,   256, 768),   # z-t0  -> pub[:, 0:512]
    (128, 256, 768),   # z-t1  -> pub[:, 0:512]
    (256, 256, 768),   # z-t2  -> pub[:, 0:512]
    (384, 384, 768),   # z-t3  -> pub[:, 128:512]
    (512, 512, 768),   # z-t4  -> pub[:, 256:512]
    (640, 640, 768),   # z-t5  -> pub[:, 384:512]
    (0,   0,   256),   # xz-t0 -> pub[:, 0:256]
    (128, 128, 256),   # xz-t1 -> pub[:, 128:256]
]
LT_OFF = []
_o = 0
for _k0, _i0, _i1 in LT_SEGS:
    LT_OFF.append(_o)
    _o += _i1 - _i0
LT_W = _o  # 2688

_cache = {}


def _build_program():
    import concourse.bacc as bacc
    import concourse.tile as tile
    from concourse import mybir

    f16 = mybir.dt.float16
    f32 = mybir.dt.float32

    nc = bacc.Bacc("TRN2", target_bir_lowering=False, debug=False)

    za_d = nc.dram_tensor("za", [NCHUNK, 128, 8, CHW], f16, kind="ExternalInput")
    lt_d = nc.dram_tensor("lt", [128, LT_W], f16, kind="ExternalInput")
    ap_d = nc.dram_tensor("apm", [128, 2, 256], f32, kind="ExternalInput")
    m2_d = nc.dram_tensor("m2", [128, 256], f16, kind="ExternalInput")
    out_d = nc.dram_tensor("out", [128, NCT], f32, kind="ExternalOutput")
    LT_HEAD = 1536  # lt cols for z-t0..t2 (tile 0's first three matmuls)

    with tile.TileContext(nc) as tc:
        with (
            tc.tile_pool(name="zpool", bufs=1) as zpool,
            tc.tile_pool(name="cpool", bufs=1) as cpool,
            tc.tile_pool(name="upool", bufs=3) as upool,
            tc.tile_pool(name="gpool", bufs=2) as gpool,
            tc.tile_pool(name="pub", bufs=7, space="PSUM") as pub_pool,
            tc.tile_pool(name="pwarm", bufs=1, space="PSUM") as pwarm_pool,
        ):
            # --- PE warm-up: junk matmuls with no DMA deps spin HAM to 2.4GHz
            warm = cpool.tile([128, 640], f16, tag="warm")
            nc.vector.memset(warm[:], 0.0)
            # e0: row 0 = ones, rest 0 -> lhsT for the mean2 broadcast matmul
            e0 = cpool.tile([128, 128], f16, tag="e0")
            nc.vector.memset(e0[:], 0.0)
            nc.vector.memset(e0[0:1, :], 1.0)
            pw = pwarm_pool.tile([128, 512], f32, tag="pw")
            for _ in range(NWARM):
                nc.tensor.matmul(pw[:], warm[:, 0:128], warm[:, 128:640],
                                 start=True, stop=True)

            # --- DMAs, ordered so c-tile 0's dependencies land first ---
            zc = [None] * NCHUNK
            zc0 = [None, None]
            ltt = cpool.tile([128, LT_W], f16, tag="lt")

            t = zpool.tile([128, 4, CHW], f16, tag="zc00")
            nc.sync.dma_start(out=t[:], in_=za_d.ap()[0, :, 0:4, :])
            zc0[0] = t
            nc.sync.dma_start(out=ltt[:, 0:LT_HEAD], in_=lt_d.ap()[:, 0:LT_HEAD])
            t = zpool.tile([128, 4, CHW], f16, tag="zc01")
            nc.sync.dma_start(out=t[:], in_=za_d.ap()[0, :, 4:8, :])
            zc0[1] = t
            nc.sync.dma_start(out=ltt[:, LT_HEAD:LT_W], in_=lt_d.ap()[:, LT_HEAD:LT_W])

            apm = cpool.tile([128, 2, 256], f32, tag="apm")
            nc.sync.dma_start(out=apm[:], in_=ap_d.ap()[:, :, :])
            m2t = cpool.tile([128, 256], f16, tag="m2")
            nc.sync.dma_start(out=m2t[:], in_=m2_d.ap()[:, :])

            for q in range(1, NCHUNK):
                t = zpool.tile([128, 8, CHW], f16, tag=f"zc{q}")
                nc.sync.dma_start(out=t[:], in_=za_d.ap()[q, :, :, :])
                zc[q] = t

            stag = cpool.tile([128, NCT], f32, tag="stag")

            def lhs(m, s):
                q, cl = divmod(m * 128, CHW)
                if q == 0:
                    return zc0[s // 4][:, s % 4, cl:cl + 128]
                return zc[q][:, s, cl:cl + 128]

            def seg(t):
                return ltt[:, LT_OFF[t]:LT_OFF[t] + (LT_SEGS[t][2] - LT_SEGS[t][1])]

            MM = nc.tensor.matmul
            for m in range(NCT):
                pub = pub_pool.tile([128, 512], f32, tag="pub")
                MM(pub[:, 0:512],   lhs(m, 0), seg(0), start=True, stop=False)
                MM(pub[:, 0:512],   lhs(m, 1), seg(1), start=False, stop=False)
                MM(pub[:, 0:512],   lhs(m, 2), seg(2), start=False, stop=False)
                MM(pub[:, 128:512], lhs(m, 3), seg(3), start=False, stop=False)
                MM(pub[:, 0:256],   lhs(m, 6), seg(6), start=False, stop=False)
                MM(pub[:, 128:256], lhs(m, 7), seg(7), start=False, stop=False)
                MM(pub[:, 256:512], e0[:, :], m2t[:, :], start=False, stop=False)
                MM(pub[:, 256:512], lhs(m, 4), seg(4), start=False, stop=False)
                MM(pub[:, 384:512], lhs(m, 5), seg(5), start=False, stop=True)

                u2 = upool.tile([128, 256], f32, tag="u2")
                nc.vector.tensor_add(u2[:], pub[:, 0:256], apm[:, m % 2, :])
                g = gpool.tile([128, 256], f32, tag="g")
                nc.vector.scalar_tensor_tensor(
                    out=g[:], in0=u2[:], scalar=0.0, in1=pub[:, 256:512],
                    op0=mybir.AluOpType.max, op1=mybir.AluOpType.mult,
                    accum_out=stag[:, m:m + 1])

                if m in (NCT // 2 - 1, NCT - 1):
                    h_ = 0 if m == NCT // 2 - 1 else 1
                    sl = slice(h_ * (NCT // 2), (h_ + 1) * (NCT // 2))
                    nc.sync.dma_start(out=out_d.ap()[:, sl], in_=stag[:, sl])

    nc.compile()
    return nc


def _prep_inputs(x, mean, cov_vector, z):
    f16 = np.float16

    L = np.zeros((P, P), dtype=np.float32)
    L[np.tril_indices(P)] = cov_vector
    d = np.diag(L).copy()
    L[np.diag_indices(P)] = np.exp(d)
    LT = L.T  # lt[k, i] = L[i, k]

    ltimg = np.empty((128, LT_W), dtype=f16)
    for (k0, i0, i1), off in zip(LT_SEGS, LT_OFF):
        ltimg[:, off:off + (i1 - i0)] = LT[k0:k0 + 128, i0:i1]

    m2img = np.zeros((128, 256), dtype=f16)
    m2img[0, :] = mean[512:768].astype(f16)

    z2 = z.reshape(P, S, B)
    in_maps = []
    for c in range(NCORES):
        zs = z2[:, :, c * BC:(c + 1) * BC].reshape(P, NCOL)  # [769, 4096] f32
        xs = x[c * BC:(c + 1) * BC]                           # [256]
        xcol = np.tile(xs, S)                                 # x per column

        zap = np.empty((NCHUNK, 128, 8, CHW), dtype=f16)
        zap[:, :, 0:6, :] = (
            zs[:768].astype(f16).reshape(6, 128, NCHUNK, CHW).transpose(2, 1, 0, 3))
        xz = (xcol[None, :] * zs[0:256]).astype(f16)          # [256, 4096]
        zap[:, :, 6, :] = xz[0:128].reshape(128, NCHUNK, CHW).transpose(1, 0, 2)
        zap[:, :, 7, :] = xz[128:256].reshape(128, NCHUNK, CHW).transpose(1, 0, 2)

        apm = (xs.reshape(2, 128).T[:, :, None] * mean[None, None, 0:256]
               + mean[None, None, 256:512]).astype(np.float32)  # [128, 2, 256]

        # b1 row handled fully on host: s3[c] = L[768,:] @ z + mean[768]
        s3 = LT[:, 768] @ zs + mean[768]                       # [4096]
        s3img = s3.reshape(NCT, 128).T.astype(np.float32)      # [128, NCT]

        in_maps.append({"za": zap, "lt": ltimg, "apm": np.ascontiguousarray(apm),
                        "m2": m2img, "_s3": s3img})
    return in_maps


def _assemble(results, s3imgs):
    out = np.empty((S, B), dtype=np.float32)
    for c in range(NCORES):
        o = results[c]["out"] + s3imgs[c]                    # [128, 32]
        oc = o.reshape(128, S, 2).transpose(1, 2, 0).reshape(S, BC)
        out[:, c * BC:(c + 1) * BC] = oc
    return out


def _run(inputs, trace=False, trace_kwargs=None):
    from concourse.bass_utils import run_bass_kernel_spmd

    if "prog" not in _cache:
        _cache["prog"] = _build_program()
    nc = _cache["prog"]

    in_maps = _prep_inputs(**inputs)
    s3imgs = [im.pop("_s3") for im in in_maps]
    kw = {}
    if trace:
        kw["trace"] = True
        if trace_kwargs:
            kw.update(trace_kwargs)
    res = run_bass_kernel_spmd(nc, in_maps, core_ids=list(range(NCORES)), **kw)
    return _assemble(res.results, s3imgs), res


def kernel(x, mean, cov_vector, z):
    out, _ = _run(dict(x=np.asarray(x), mean=np.asarray(mean),
                       cov_vector=np.asarray(cov_vector), z=np.asarray(z)))
    return out
